# revision 13
# baseline (speedup 1.0000x reference)
"""Trainium2 Bass kernel for nn_CDFL1HistogramLoss (CDF-L1 histogram loss).

Math (see reference): per channel (16->256 resized), the CDF numerators
telescope to T(t) = sum_x sig(C*(u_x - t)), C = SIGMA/256, u = 256*x.
Decompose u = 16*m + w (m = round(u/16) in [0,16], w in [-8,8]).  Each
pixel's sigmoid profile sig(C*(w - (t - 16m))) is expanded in a small
per-pixel basis (tanh anchors from ScalarE + w-powers from VectorE); the
device scatters basis columns by coarse bucket m with a PE one-hot
matmul (stationary = basis columns Phi [128, NCOL], moving = one-hot
[128, 17], PSUM-accumulated over all 512 pixel chunks in 4 PE column
strips).  The host folds A[m, col] through a least-squares fit L of
sig(C*(w - trel)) in the (bf16-quantized) device basis to get T, the
CDFs, and the loss in float64.

Sharding: data-parallel over batch: core i handles batches [2i, 2i+1]
(12 channel-histograms = 6 pred + 6 target per core).
"""
import os
import numpy as np

import concourse.bass as bass
import concourse.bacc as bacc
import concourse.mybir as mybir
from concourse import tile
from concourse.bass_utils import run_bass_kernel_spmd

F32 = mybir.dt.float32
BF16 = mybir.dt.bfloat16
I32 = mybir.dt.int32
ALU = mybir.AluOpType
ACT = mybir.ActivationFunctionType

N_CORES = 8
SIGMA = 300.0
C = SIGMA / 256.0             # z-units per fine bin (1.171875)
N_M = 17                      # coarse buckets m = round(u/16) in [0, 16]
NPIX = 65536

# ---- basis definition (device + host must agree) ----
ANCHORS = (-10.5, -7.5, -4.5, -1.5, 1.5, 4.5, 7.5, 10.5)   # fine-bin offsets
N_POW = 3                     # wn, wn^2, wn^3 with wn = w/8 in [-1, 1]
N_A = len(ANCHORS)
NCOL = N_A + N_POW + 1        # + ones column (counts); must be <= 32


def bf16_rne(x):
    f = np.asarray(x, np.float32)
    u = f.view(np.uint32)
    r = ((u >> 16) & 1) + 0x7FFF
    return ((u + r) & 0xFFFF0000).view(np.float32)


def make_mh2() -> np.ndarray:
    """[512, 256] vertical resize matrix x2 (so v = 3a + b needs no extra scale)."""
    M = np.zeros((512, 256), dtype=np.float64)
    for i in range(256):
        if i == 0:
            M[0, 0], M[1, 0], M[2, 0] = 3/7, 3/7, 1/7
        elif i == 255:
            M[509, 255], M[510, 255], M[511, 255] = 1/7, 3/7, 3/7
        else:
            M[2*i-1, i] = 1/8; M[2*i, i] = 3/8; M[2*i+1, i] = 3/8; M[2*i+2, i] = 1/8
    return (2.0 * M).astype(np.float32)


def device_basis(wv):
    """Device-accurate basis columns for wv (f32 array in [-0.5, 0.5]).

    Column order matches the Phi SBUF tile rows:
      0..N_A-1 : bf16(tanh(8C*wv - (C/2)*t_r))
      N_A + p  : wn^(p+1) chain in bf16, wn = bf16(2*wv)
      NCOL-1   : ones
    """
    wv = np.asarray(wv, np.float32)
    cols = []
    for t in ANCHORS:
        cols.append(bf16_rne(np.tanh(np.float32(8.0*C)*wv + np.float32(-(C/2.0)*t),
                                     dtype=np.float64).astype(np.float32)))
    wn = bf16_rne(2.0*wv)
    p = wn
    cols.append(p)
    for _ in range(N_POW - 1):
        p = bf16_rne(p * wn)
        cols.append(p)
    cols.append(np.ones_like(wv))
    return np.stack(cols, axis=-1)   # [..., NCOL]


def host_fit():
    """L[col, trel+256] fitting sig(C*(16*wv - trel)) in the device basis."""
    wv = np.linspace(-0.5, 0.5, 8193).astype(np.float32)
    B = device_basis(wv).astype(np.float64)
    trels = np.arange(-256, 257)
    G = 1.0/(1.0 + np.exp(-C*(16.0*wv.astype(np.float64)[:, None] - trels[None, :])))
    L, *_ = np.linalg.lstsq(B, G, rcond=None)
    return L                      # [NCOL, 513]


def build(n_pairs: int = 6):
    """Per-core Bass program: n_pairs pred + n_pairs target channels."""
    MH2 = make_mh2()
    # nonzero 128x128 blocks of MH2 per output half
    mh_blocks = {}
    for half in range(2):
        qs = []
        for q in range(4):
            if np.any(MH2[128*q:128*(q+1), 128*half:128*(half+1)] != 0):
                qs.append(q)
        mh_blocks[half] = qs
    n_ch = 2 * n_pairs

    GP = 8                      # chunks packed per scatter matmul (diagonal blocks)
    nc = bacc.Bacc("TRN2", target_bir_lowering=False, debug=False, num_devices=N_CORES)
    pred = nc.dram_tensor("pred", [2, 3, 512, 512], F32, kind="ExternalInput").ap()
    target = nc.dram_tensor("target", [2, 3, 512, 512], F32, kind="ExternalInput").ap()
    mh = nc.dram_tensor("mh", [512, 256], F32, kind="ExternalInput").ap()
    out = nc.dram_tensor("out", [n_ch, GP * NCOL, GP * N_M], F32,
                         kind="ExternalOutput").ap()

    with tile.TileContext(nc) as tc:
        from contextlib import ExitStack
        nv = nc.vector
        ns = nc.scalar
        ctx = ExitStack()
        cpool = ctx.enter_context(tc.tile_pool(name="consts", bufs=1))

        mh_sb = cpool.tile(shape=[128, 4, 256], dtype=F32, name="mh_sb")
        nc.sync.dma_start(mh_sb, mh.rearrange("(q p) w -> p q w", p=128))
        # bias_sb[:, r] = -(C/2)*ANCHORS[r]  (anchors uniform: -10.5 + 3r)
        iota_i = cpool.tile(shape=[128, 32], dtype=I32, name="iota_i")
        nc.gpsimd.iota(iota_i, pattern=[[1, 32]], base=0, channel_multiplier=0)
        bias_sb = cpool.tile(shape=[128, N_A], dtype=F32, name="bias_sb")
        nv.tensor_scalar(bias_sb, iota_i[:, 0:N_A], float(-3.0*(C/2.0)),
                         float(10.5*(C/2.0)), ALU.mult, ALU.add)

        ch_ctx = ExitStack()
        io_pool = ch_ctx.enter_context(tc.tile_pool(name="io", bufs=3))
        wk_pool = ch_ctx.enter_context(tc.tile_pool(name="wk", bufs=2))
        phi_pool = ch_ctx.enter_context(tc.tile_pool(name="phi", bufs=2))
        oh_pool = ch_ctx.enter_context(tc.tile_pool(name="oh", bufs=2))
        hp_pool = ch_ctx.enter_context(tc.tile_pool(name="hp", bufs=2, space="PSUM"))
        at_pool = ch_ctx.enter_context(tc.tile_pool(name="at", bufs=2, space="PSUM"))

        chans = [("p", i) for i in range(n_pairs)] + [("t", i) for i in range(n_pairs)]

        for ci, (grp, pi) in enumerate(chans):
            b, cch = divmod(pi, 3)
            src = (pred if grp == "p" else target)[b, cch]      # [512, 512] dram
            raw = io_pool.tile(shape=[128, 4, 512], dtype=F32, name="raw")
            # alternate the two HWDGE rings (sync / scalar) for input BW
            (nc.sync if ci % 2 == 0 else nc.scalar).dma_start(
                raw, src.rearrange("(q p) w -> p q w", p=128))

            hs = wk_pool.tile(shape=[128, 2, 512], dtype=F32, name="hs")
            for half in range(2):
                hp = hp_pool.tile(shape=[128, 512], dtype=F32, space="PSUM", name="hp")
                qs = mh_blocks[half]
                for qi, q in enumerate(qs):
                    nc.tensor.matmul(
                        hp, mh_sb[:, q, 128*half:128*(half+1)], raw[:, q, :],
                        start=(qi == 0), stop=(qi == len(qs) - 1),
                    )
                # PSUM -> SBUF (one per engine to balance ACT/DVE)
                if half == 0:
                    ns.copy(hs[:, half], hp)
                else:
                    nv.tensor_copy(hs[:, half], hp)

            # horizontal resize: v = 16*y in [0, 16]
            v = wk_pool.tile(shape=[128, 2, 256], dtype=F32, name="v")
            a2 = wk_pool.tile(shape=[128, 2, 256], dtype=F32, name="a2")
            b2 = wk_pool.tile(shape=[128, 2, 256], dtype=F32, name="b2")
            hsr = hs.rearrange("p h (i two) -> p h i two", two=2)
            ev, od = hsr[:, :, :, 0], hsr[:, :, :, 1]
            nv.tensor_tensor(a2, ev, od, ALU.add)                      # [128,2,256]
            nv.tensor_tensor(b2[:, :, 1:255], od[:, :, 0:254], ev[:, :, 2:256], ALU.add)
            nv.scalar_tensor_tensor(v[:, :, 1:255], a2[:, :, 1:255], 3.0,
                                    b2[:, :, 1:255], ALU.mult, ALU.add)
            for half in range(2):
                # v[0] = (8/7)*(3*a2[0] + hs[2]); v[255] = (8/7)*(3*a2[255] + hs[509])
                nv.scalar_tensor_tensor(v[:, half, 0:1], a2[:, half, 0:1], 3.0,
                                        hs[:, half, 2:3], ALU.mult, ALU.add)
                nv.tensor_scalar(v[:, half, 0:1], v[:, half, 0:1], 8.0/7.0, None, ALU.mult)
                nv.scalar_tensor_tensor(v[:, half, 255:256], a2[:, half, 255:256], 3.0,
                                        hs[:, half, 509:510], ALU.mult, ALU.add)
                nv.tensor_scalar(v[:, half, 255:256], v[:, half, 255:256], 8.0/7.0,
                                 None, ALU.mult)

            vf = v.rearrange("p h i -> p (h i)")                       # [128, 512]
            # h = round(v) in [0, 16] (DVE f32->i32 convert rounds to nearest)
            h32 = wk_pool.tile(shape=[128, 512], dtype=I32, name="h32")
            nv.tensor_copy(h32, vf)
            hbf = wk_pool.tile(shape=[128, 512], dtype=BF16, name="hbf")
            nv.tensor_copy(hbf, h32)
            # wv = v - h in [-0.5, 0.5] (mixed f32/bf16 operands; h exact in bf16)
            wv = wk_pool.tile(shape=[128, 512], dtype=F32, name="wv")
            nv.tensor_tensor(wv, vf, hbf, ALU.subtract)

            # basis columns Phi [128, NCOL, 512] bf16
            phi = phi_pool.tile(shape=[128, NCOL, 512], dtype=BF16, name="phi")
            for r in range(N_A):
                ns.activation(phi[:, r, :], wv, ACT.Tanh,
                              bias=bias_sb[:, r:r+1], scale=float(8.0*C))
            wn = wk_pool.tile(shape=[128, 512], dtype=BF16, name="wn")
            nv.tensor_scalar(wn, wv, 2.0, None, ALU.mult)
            nv.tensor_copy(phi[:, N_A, :], wn)
            for p in range(1, N_POW):
                nv.tensor_tensor(phi[:, N_A+p, :], phi[:, N_A+p-1, :], wn, ALU.mult)
            nv.memset(phi[:, NCOL-1, :], 1.0)

            # one-hot [128, 17, 512] bf16 (split across DVE and idle GpSimd)
            oh = oh_pool.tile(shape=[128, N_M, 512], dtype=BF16, name="oh")
            for m in range(N_M):
                eng = nv if m < 11 else nc.gpsimd
                eng.tensor_scalar(oh[:, m, :], hbf, float(m), None, ALU.is_equal)

            # scatter: GP=8 chunks per matmul.  Group g packs chunks
            # {g, g+64, .., g+448}: with columns ordered (c outer, j inner)
            # the address c*512 + 64*j is affine with stride 64, so both
            # operands collapse to a single legal free dim while phi/oh keep
            # their fast-write layouts.  Output cell (8c+j, 8m+j') accumulates
            # phi_c^T oh_m of chunk pair (j, j'); only the j==j' diagonal is
            # read on the host.
            aps = at_pool.tile(shape=[128, 512], dtype=F32, space="PSUM", name="aps")
            n_groups = 512 // GP
            for g in range(n_groups):
                st = phi[:, :, g::n_groups].rearrange("p c j -> p (c j)")
                mv = oh[:, :, g::n_groups].rearrange("p m j -> p (m j)")
                nc.tensor.matmul(aps[0:GP*NCOL, 0:GP*N_M], st, mv,
                                 start=(g == 0), stop=(g == n_groups - 1))
            stage = wk_pool.tile(shape=[GP*NCOL, GP*N_M], dtype=F32, name="stage")
            ns.copy(stage, aps[0:GP*NCOL, 0:GP*N_M])
            (nc.sync if ci % 2 == 0 else nc.scalar).dma_start(out[ci], stage)

        ch_ctx.close()
        ctx.close()

    nc.compile()
    return nc


_CACHE: dict = {}
LAST_RESULT = None


def _get_nc(n_pairs=6):
    if n_pairs not in _CACHE:
        _CACHE[n_pairs] = build(n_pairs)
    return _CACHE[n_pairs]


def kernel(pred: np.ndarray, target: np.ndarray) -> np.ndarray:
    global LAST_RESULT
    pred = np.ascontiguousarray(pred, dtype=np.float32)
    target = np.ascontiguousarray(target, dtype=np.float32)
    assert pred.shape == (16, 3, 512, 512) and target.shape == (16, 3, 512, 512)

    nc = _get_nc(6)
    mh_buf = make_mh2()
    in_maps = []
    for i in range(N_CORES):
        in_maps.append({
            "pred": pred[2*i:2*i+2],
            "target": target[2*i:2*i+2],
            "mh": mh_buf,
        })
    trace = os.environ.get("KERNEL_TRACE", "0") == "1"
    res = run_bass_kernel_spmd(nc, in_maps, core_ids=list(range(N_CORES)), trace=trace)
    LAST_RESULT = res

    # host-side fold: A[col, m] -> T(tp) -> CDF -> loss (float64)
    GP = 8
    L = host_fit()                                    # [NCOL, 513]
    tps = np.arange(257)
    losses = []
    for i in range(N_CORES):
        a = res.results[i]["out"].astype(np.float64)  # [12, GP*NCOL, GP*17]
        a4 = a.reshape(12, NCOL, GP, N_M, GP)
        A = np.diagonal(a4, axis1=2, axis2=4).sum(axis=-1)   # [12, NCOL, 17]
        cdfs = []
        for ch in range(12):
            T = np.zeros(257)
            for m in range(N_M):
                T += (A[ch, :, m][:, None] * L[:, tps - 16*m + 256]).sum(axis=0)
            Cn = T[0] - T[1:]
            cdfs.append(Cn / Cn[-1])
        for p in range(6):
            losses.append(np.mean(np.abs(cdfs[p] - cdfs[p+6])))
    return np.float32(np.mean(losses))


# revision 14
# speedup vs baseline: 2.3199x; 2.3199x over previous
"""Trainium2 Bass kernel for nn_CDFL1HistogramLoss (CDF-L1 histogram loss).

Math (see reference): per channel (16->256 resized), the CDF numerators
telescope to T(t) = sum_x sig(C*(u_x - t)), C = SIGMA/256, u = 256*x.
Decompose u = 16*m + w (m = round(u/16) in [0,16], w in [-8,8]).  Each
pixel's sigmoid profile sig(C*(w - (t - 16m))) is expanded in a small
per-pixel basis (tanh anchors from ScalarE + w-powers from VectorE); the
device scatters basis columns by coarse bucket m with a PE one-hot
matmul (stationary = basis columns Phi [128, NCOL], moving = one-hot
[128, 17], PSUM-accumulated over all 512 pixel chunks in 4 PE column
strips).  The host folds A[m, col] through a least-squares fit L of
sig(C*(w - trel)) in the (bf16-quantized) device basis to get T, the
CDFs, and the loss in float64.

Sharding: data-parallel over batch: core i handles batches [2i, 2i+1]
(12 channel-histograms = 6 pred + 6 target per core).
"""
import os
import numpy as np

import concourse.bass as bass
import concourse.bacc as bacc
import concourse.mybir as mybir
from concourse import tile
from concourse.bass_utils import run_bass_kernel_spmd

F32 = mybir.dt.float32
BF16 = mybir.dt.bfloat16
I32 = mybir.dt.int32
ALU = mybir.AluOpType
ACT = mybir.ActivationFunctionType

N_CORES = 8
SIGMA = 300.0
C = SIGMA / 256.0             # z-units per fine bin (1.171875)
N_M = 17                      # coarse buckets m = round(u/16) in [0, 16]
NPIX = 65536

# ---- basis definition (device + host must agree) ----
ANCHORS = (-10.5, -7.5, -4.5, -1.5, 1.5, 4.5, 7.5, 10.5)   # fine-bin offsets
N_POW = 3                     # wn, wn^2, wn^3 with wn = w/8 in [-1, 1]
N_A = len(ANCHORS)
NCOL = N_A + N_POW + 1        # + ones column (counts); must be <= 32


def bf16_rne(x):
    f = np.asarray(x, np.float32)
    u = f.view(np.uint32)
    r = ((u >> 16) & 1) + 0x7FFF
    return ((u + r) & 0xFFFF0000).view(np.float32)


def make_mh2() -> np.ndarray:
    """[512, 256] vertical resize matrix x2 (so v = 3a + b needs no extra scale)."""
    M = np.zeros((512, 256), dtype=np.float64)
    for i in range(256):
        if i == 0:
            M[0, 0], M[1, 0], M[2, 0] = 3/7, 3/7, 1/7
        elif i == 255:
            M[509, 255], M[510, 255], M[511, 255] = 1/7, 3/7, 3/7
        else:
            M[2*i-1, i] = 1/8; M[2*i, i] = 3/8; M[2*i+1, i] = 3/8; M[2*i+2, i] = 1/8
    return (2.0 * M).astype(np.float32)


def device_basis(wv):
    """Device-accurate basis columns for wv (f32 array in [-0.5, 0.5]).

    Column order matches the Phi SBUF tile rows:
      0..N_A-1 : bf16(tanh(8C*wv - (C/2)*t_r))
      N_A + p  : wn^(p+1) chain in bf16, wn = bf16(2*wv)
      NCOL-1   : ones
    """
    wv = np.asarray(wv, np.float32)
    cols = []
    for t in ANCHORS:
        cols.append(bf16_rne(np.tanh(np.float32(8.0*C)*wv + np.float32(-(C/2.0)*t),
                                     dtype=np.float64).astype(np.float32)))
    wn = bf16_rne(2.0*wv)
    p = wn
    cols.append(p)
    for _ in range(N_POW - 1):
        p = bf16_rne(p * wn)
        cols.append(p)
    cols.append(np.ones_like(wv))
    return np.stack(cols, axis=-1)   # [..., NCOL]


def host_fit():
    """L[col, trel+256] fitting sig(C*(16*wv - trel)) in the device basis."""
    wv = np.linspace(-0.5, 0.5, 8193).astype(np.float32)
    B = device_basis(wv).astype(np.float64)
    trels = np.arange(-256, 257)
    G = 1.0/(1.0 + np.exp(-C*(16.0*wv.astype(np.float64)[:, None] - trels[None, :])))
    L, *_ = np.linalg.lstsq(B, G, rcond=None)
    return L                      # [NCOL, 513]


def build(n_pairs: int = 6):
    """Per-core Bass program: n_pairs pred + n_pairs target channels."""
    MH2 = make_mh2()
    # nonzero 128x128 blocks of MH2 per output half
    mh_blocks = {}
    for half in range(2):
        qs = []
        for q in range(4):
            if np.any(MH2[128*q:128*(q+1), 128*half:128*(half+1)] != 0):
                qs.append(q)
        mh_blocks[half] = qs
    n_ch = 2 * n_pairs

    GP = 8                      # chunks packed per scatter matmul (diagonal blocks)
    nc = bacc.Bacc("TRN2", target_bir_lowering=False, debug=False, num_devices=N_CORES)
    pred = nc.dram_tensor("pred", [2, 3, 512, 512], F32, kind="ExternalInput").ap()
    target = nc.dram_tensor("target", [2, 3, 512, 512], F32, kind="ExternalInput").ap()
    mh = nc.dram_tensor("mh", [512, 256], F32, kind="ExternalInput").ap()
    out = nc.dram_tensor("out", [n_ch, GP * NCOL, GP * N_M], F32,
                         kind="ExternalOutput").ap()

    with tile.TileContext(nc) as tc:
        from contextlib import ExitStack
        nv = nc.vector
        ns = nc.scalar
        ctx = ExitStack()
        cpool = ctx.enter_context(tc.tile_pool(name="consts", bufs=1))

        mh_sb = cpool.tile(shape=[128, 4, 256], dtype=F32, name="mh_sb")
        nc.sync.dma_start(mh_sb, mh.rearrange("(q p) w -> p q w", p=128))
        # bias_sb[:, r] = -(C/2)*ANCHORS[r]  (anchors uniform: -10.5 + 3r)
        iota_i = cpool.tile(shape=[128, 32], dtype=I32, name="iota_i")
        nc.gpsimd.iota(iota_i, pattern=[[1, 32]], base=0, channel_multiplier=0)
        bias_sb = cpool.tile(shape=[128, N_A], dtype=F32, name="bias_sb")
        nv.tensor_scalar(bias_sb, iota_i[:, 0:N_A], float(-3.0*(C/2.0)),
                         float(10.5*(C/2.0)), ALU.mult, ALU.add)

        ch_ctx = ExitStack()
        io_pool = ch_ctx.enter_context(tc.tile_pool(name="io", bufs=3))
        wk_pool = ch_ctx.enter_context(tc.tile_pool(name="wk", bufs=2))
        phi_pool = ch_ctx.enter_context(tc.tile_pool(name="phi", bufs=2))
        oh_pool = ch_ctx.enter_context(tc.tile_pool(name="oh", bufs=2))
        hp_pool = ch_ctx.enter_context(tc.tile_pool(name="hp", bufs=2, space="PSUM"))
        at_pool = ch_ctx.enter_context(tc.tile_pool(name="at", bufs=2, space="PSUM"))

        chans = [("p", i) for i in range(n_pairs)] + [("t", i) for i in range(n_pairs)]

        for ci, (grp, pi) in enumerate(chans):
            b, cch = divmod(pi, 3)
            src = (pred if grp == "p" else target)[b, cch]      # [512, 512] dram
            raw = io_pool.tile(shape=[128, 4, 512], dtype=F32, name="raw")
            # alternate the two HWDGE rings (sync / scalar) for input BW
            (nc.sync if ci % 2 == 0 else nc.scalar).dma_start(
                raw, src.rearrange("(q p) w -> p q w", p=128))

            hs = wk_pool.tile(shape=[128, 2, 512], dtype=F32, name="hs")
            for half in range(2):
                hp = hp_pool.tile(shape=[128, 512], dtype=F32, space="PSUM", name="hp")
                qs = mh_blocks[half]
                for qi, q in enumerate(qs):
                    nc.tensor.matmul(
                        hp, mh_sb[:, q, 128*half:128*(half+1)], raw[:, q, :],
                        start=(qi == 0), stop=(qi == len(qs) - 1),
                    )
                # PSUM -> SBUF (one per engine to balance ACT/DVE)
                if half == 0:
                    ns.copy(hs[:, half], hp)
                else:
                    nv.tensor_copy(hs[:, half], hp)

            # horizontal resize: v = 16*y in [0, 16]
            v = wk_pool.tile(shape=[128, 2, 256], dtype=F32, name="v")
            a2 = wk_pool.tile(shape=[128, 2, 256], dtype=F32, name="a2")
            b2 = wk_pool.tile(shape=[128, 2, 256], dtype=F32, name="b2")
            hsr = hs.rearrange("p h (i two) -> p h i two", two=2)
            ev, od = hsr[:, :, :, 0], hsr[:, :, :, 1]
            nv.tensor_tensor(a2, ev, od, ALU.add)                      # [128,2,256]
            nv.tensor_tensor(b2[:, :, 1:255], od[:, :, 0:254], ev[:, :, 2:256], ALU.add)
            nv.scalar_tensor_tensor(v[:, :, 1:255], a2[:, :, 1:255], 3.0,
                                    b2[:, :, 1:255], ALU.mult, ALU.add)
            for half in range(2):
                # v[0] = (8/7)*(3*a2[0] + hs[2]); v[255] = (8/7)*(3*a2[255] + hs[509])
                nv.scalar_tensor_tensor(v[:, half, 0:1], a2[:, half, 0:1], 3.0,
                                        hs[:, half, 2:3], ALU.mult, ALU.add)
                nv.tensor_scalar(v[:, half, 0:1], v[:, half, 0:1], 8.0/7.0, None, ALU.mult)
                nv.scalar_tensor_tensor(v[:, half, 255:256], a2[:, half, 255:256], 3.0,
                                        hs[:, half, 509:510], ALU.mult, ALU.add)
                nv.tensor_scalar(v[:, half, 255:256], v[:, half, 255:256], 8.0/7.0,
                                 None, ALU.mult)

            vf = v.rearrange("p h i -> p (h i)")                       # [128, 512]
            # h = round(v) in [0, 16] (DVE f32->i32 convert rounds to nearest)
            h32 = wk_pool.tile(shape=[128, 512], dtype=I32, name="h32")
            nv.tensor_copy(h32, vf)
            hbf = wk_pool.tile(shape=[128, 512], dtype=BF16, name="hbf")
            nv.tensor_copy(hbf, h32)
            # wv = v - h in [-0.5, 0.5] (mixed f32/bf16 operands; h exact in bf16)
            wv = wk_pool.tile(shape=[128, 512], dtype=F32, name="wv")
            nv.tensor_tensor(wv, vf, hbf, ALU.subtract)

            # basis columns Phi [128, NCOL, 512] bf16
            phi = phi_pool.tile(shape=[128, NCOL, 512], dtype=BF16, name="phi")
            for r in range(N_A):
                ns.activation(phi[:, r, :], wv, ACT.Tanh,
                              bias=bias_sb[:, r:r+1], scale=float(8.0*C))
            wn = wk_pool.tile(shape=[128, 512], dtype=BF16, name="wn")
            nv.tensor_scalar(wn, wv, 2.0, None, ALU.mult)
            nv.tensor_copy(phi[:, N_A, :], wn)
            for p in range(1, N_POW):
                nv.tensor_tensor(phi[:, N_A+p, :], phi[:, N_A+p-1, :], wn, ALU.mult)
            nv.memset(phi[:, NCOL-1, :], 1.0)

            # one-hot [128, 17, 512] bf16 (DVE only: GpSimd shares the SBUF
            # port with DVE under an exclusive lock — offloading there slows
            # both engines ~10x)
            oh = oh_pool.tile(shape=[128, N_M, 512], dtype=BF16, name="oh")
            for m in range(N_M):
                nv.tensor_scalar(oh[:, m, :], hbf, float(m), None, ALU.is_equal)

            # scatter: GP=8 chunks per matmul.  Group g packs chunks
            # {g, g+64, .., g+448}: with columns ordered (c outer, j inner)
            # the address c*512 + 64*j is affine with stride 64, so both
            # operands collapse to a single legal free dim while phi/oh keep
            # their fast-write layouts.  Output cell (8c+j, 8m+j') accumulates
            # phi_c^T oh_m of chunk pair (j, j'); only the j==j' diagonal is
            # read on the host.
            aps = at_pool.tile(shape=[128, 512], dtype=F32, space="PSUM", name="aps")
            n_groups = 512 // GP
            for g in range(n_groups):
                st = phi[:, :, g::n_groups].rearrange("p c j -> p (c j)")
                mv = oh[:, :, g::n_groups].rearrange("p m j -> p (m j)")
                nc.tensor.matmul(aps[0:GP*NCOL, 0:GP*N_M], st, mv,
                                 start=(g == 0), stop=(g == n_groups - 1))
            stage = wk_pool.tile(shape=[GP*NCOL, GP*N_M], dtype=F32, name="stage")
            ns.copy(stage, aps[0:GP*NCOL, 0:GP*N_M])
            (nc.sync if ci % 2 == 0 else nc.scalar).dma_start(out[ci], stage)

        ch_ctx.close()
        ctx.close()

    nc.compile()
    return nc


_CACHE: dict = {}
LAST_RESULT = None


def _get_nc(n_pairs=6):
    if n_pairs not in _CACHE:
        _CACHE[n_pairs] = build(n_pairs)
    return _CACHE[n_pairs]


def kernel(pred: np.ndarray, target: np.ndarray) -> np.ndarray:
    global LAST_RESULT
    pred = np.ascontiguousarray(pred, dtype=np.float32)
    target = np.ascontiguousarray(target, dtype=np.float32)
    assert pred.shape == (16, 3, 512, 512) and target.shape == (16, 3, 512, 512)

    nc = _get_nc(6)
    mh_buf = make_mh2()
    in_maps = []
    for i in range(N_CORES):
        in_maps.append({
            "pred": pred[2*i:2*i+2],
            "target": target[2*i:2*i+2],
            "mh": mh_buf,
        })
    trace = os.environ.get("KERNEL_TRACE", "0") == "1"
    res = run_bass_kernel_spmd(nc, in_maps, core_ids=list(range(N_CORES)), trace=trace)
    LAST_RESULT = res

    # host-side fold: A[col, m] -> T(tp) -> CDF -> loss (float64)
    GP = 8
    L = host_fit()                                    # [NCOL, 513]
    tps = np.arange(257)
    losses = []
    for i in range(N_CORES):
        a = res.results[i]["out"].astype(np.float64)  # [12, GP*NCOL, GP*17]
        a4 = a.reshape(12, NCOL, GP, N_M, GP)
        A = np.diagonal(a4, axis1=2, axis2=4).sum(axis=-1)   # [12, NCOL, 17]
        cdfs = []
        for ch in range(12):
            T = np.zeros(257)
            for m in range(N_M):
                T += (A[ch, :, m][:, None] * L[:, tps - 16*m + 256]).sum(axis=0)
            Cn = T[0] - T[1:]
            cdfs.append(Cn / Cn[-1])
        for p in range(6):
            losses.append(np.mean(np.abs(cdfs[p] - cdfs[p+6])))
    return np.float32(np.mean(losses))


# revision 17
# speedup vs baseline: 2.3551x; 1.0152x over previous
"""Trainium2 Bass kernel for nn_CDFL1HistogramLoss (CDF-L1 histogram loss).

Math (see reference): per channel (16->256 resized), the CDF numerators
telescope to T(t) = sum_x sig(C*(u_x - t)), C = SIGMA/256, u = 256*x.
Decompose u = 16*m + w (m = round(u/16) in [0,16], w in [-8,8]).  Each
pixel's sigmoid profile sig(C*(w - (t - 16m))) is expanded in a small
per-pixel basis (tanh anchors from ScalarE + w-powers from VectorE); the
device scatters basis columns by coarse bucket m with a PE one-hot
matmul (stationary = basis columns Phi [128, NCOL], moving = one-hot
[128, 17], PSUM-accumulated over all 512 pixel chunks in 4 PE column
strips).  The host folds A[m, col] through a least-squares fit L of
sig(C*(w - trel)) in the (bf16-quantized) device basis to get T, the
CDFs, and the loss in float64.

Sharding: data-parallel over batch: core i handles batches [2i, 2i+1]
(12 channel-histograms = 6 pred + 6 target per core).
"""
import os
import numpy as np

import concourse.bass as bass
import concourse.bacc as bacc
import concourse.mybir as mybir
from concourse import tile
from concourse.bass_utils import run_bass_kernel_spmd

F32 = mybir.dt.float32
BF16 = mybir.dt.bfloat16
I32 = mybir.dt.int32
ALU = mybir.AluOpType
ACT = mybir.ActivationFunctionType

N_CORES = 8
SIGMA = 300.0
C = SIGMA / 256.0             # z-units per fine bin (1.171875)
N_M = 17                      # coarse buckets m = round(u/16) in [0, 16]
NPIX = 65536

# ---- basis definition (device + host must agree) ----
ANCHORS = (-10.5, -7.5, -4.5, -1.5, 1.5, 4.5, 7.5, 10.5)   # fine-bin offsets
N_POW = 3                     # wn, wn^2, wn^3 with wn = w/8 in [-1, 1]
N_A = len(ANCHORS)
NCOL = N_A + N_POW + 1        # + ones column (counts); must be <= 32


def bf16_rne(x):
    f = np.asarray(x, np.float32)
    u = f.view(np.uint32)
    r = ((u >> 16) & 1) + 0x7FFF
    return ((u + r) & 0xFFFF0000).view(np.float32)


def make_mh2() -> np.ndarray:
    """[512, 256] vertical resize matrix x2 (so v = 3a + b needs no extra scale)."""
    M = np.zeros((512, 256), dtype=np.float64)
    for i in range(256):
        if i == 0:
            M[0, 0], M[1, 0], M[2, 0] = 3/7, 3/7, 1/7
        elif i == 255:
            M[509, 255], M[510, 255], M[511, 255] = 1/7, 3/7, 3/7
        else:
            M[2*i-1, i] = 1/8; M[2*i, i] = 3/8; M[2*i+1, i] = 3/8; M[2*i+2, i] = 1/8
    return (2.0 * M).astype(np.float32)


def device_basis(wv):
    """Device-accurate basis columns for wv (f32 array in [-0.5, 0.5]).

    Column order matches the Phi SBUF tile rows:
      0..N_A-1 : bf16(tanh(8C*wv - (C/2)*t_r))
      N_A + p  : wn^(p+1) chain in bf16, wn = bf16(2*wv)
      NCOL-1   : ones
    """
    wv = np.asarray(wv, np.float32)
    cols = []
    for t in ANCHORS:
        cols.append(bf16_rne(np.tanh(np.float32(8.0*C)*wv + np.float32(-(C/2.0)*t),
                                     dtype=np.float64).astype(np.float32)))
    wn = bf16_rne(2.0*wv)
    p = wn
    cols.append(p)
    for _ in range(N_POW - 1):
        p = bf16_rne(p * wn)
        cols.append(p)
    cols.append(np.ones_like(wv))
    return np.stack(cols, axis=-1)   # [..., NCOL]


def host_fit():
    """L[col, trel+256] fitting sig(C*(16*wv - trel)) in the device basis."""
    wv = np.linspace(-0.5, 0.5, 8193).astype(np.float32)
    B = device_basis(wv).astype(np.float64)
    trels = np.arange(-256, 257)
    G = 1.0/(1.0 + np.exp(-C*(16.0*wv.astype(np.float64)[:, None] - trels[None, :])))
    L, *_ = np.linalg.lstsq(B, G, rcond=None)
    return L                      # [NCOL, 513]


def build(n_pairs: int = 6):
    """Per-core Bass program: n_pairs pred + n_pairs target channels."""
    MH2 = make_mh2()
    # nonzero 128x128 blocks of MH2 per output half
    mh_blocks = {}
    for half in range(2):
        qs = []
        for q in range(4):
            if np.any(MH2[128*q:128*(q+1), 128*half:128*(half+1)] != 0):
                qs.append(q)
        mh_blocks[half] = qs
    n_ch = 2 * n_pairs

    GP = 8                      # chunks packed per scatter matmul (diagonal blocks)
    nc = bacc.Bacc("TRN2", target_bir_lowering=False, debug=False, num_devices=N_CORES)
    pred = nc.dram_tensor("pred", [2, 3, 512, 512], F32, kind="ExternalInput").ap()
    target = nc.dram_tensor("target", [2, 3, 512, 512], F32, kind="ExternalInput").ap()
    mh = nc.dram_tensor("mh", [512, 256], F32, kind="ExternalInput").ap()
    out = nc.dram_tensor("out", [n_ch, GP * NCOL, GP * N_M], F32,
                         kind="ExternalOutput").ap()

    with tile.TileContext(nc) as tc:
        from contextlib import ExitStack
        nv = nc.vector
        ns = nc.scalar
        ctx = ExitStack()
        cpool = ctx.enter_context(tc.tile_pool(name="consts", bufs=1))

        mh_sb = cpool.tile(shape=[128, 4, 256], dtype=F32, name="mh_sb")
        nc.sync.dma_start(mh_sb, mh.rearrange("(q p) w -> p q w", p=128))
        # bias_sb[:, r] = -(C/2)*ANCHORS[r]  (anchors uniform: -10.5 + 3r)
        iota_i = cpool.tile(shape=[128, 32], dtype=I32, name="iota_i")
        nc.gpsimd.iota(iota_i, pattern=[[1, 32]], base=0, channel_multiplier=0)
        bias_sb = cpool.tile(shape=[128, N_A], dtype=F32, name="bias_sb")
        nv.tensor_scalar(bias_sb, iota_i[:, 0:N_A], float(-3.0*(C/2.0)),
                         float(10.5*(C/2.0)), ALU.mult, ALU.add)

        ch_ctx = ExitStack()
        io_pool = ch_ctx.enter_context(tc.tile_pool(name="io", bufs=3))
        wk_pool = ch_ctx.enter_context(tc.tile_pool(name="wk", bufs=2))
        phi_pool = ch_ctx.enter_context(tc.tile_pool(name="phi", bufs=2))
        oh_pool = ch_ctx.enter_context(tc.tile_pool(name="oh", bufs=2))
        hp_pool = ch_ctx.enter_context(tc.tile_pool(name="hp", bufs=4, space="PSUM"))
        at_pool = ch_ctx.enter_context(tc.tile_pool(name="at", bufs=2, space="PSUM"))

        chans = [("p", i) for i in range(n_pairs)] + [("t", i) for i in range(n_pairs)]
        state: dict = {}

        def stage_dma(ci):
            grp, pi = chans[ci]
            b, cch = divmod(pi, 3)
            src = (pred if grp == "p" else target)[b, cch]      # [512, 512] dram
            raw = io_pool.tile(shape=[128, 4, 512], dtype=F32, name="raw")
            # alternate the two HWDGE rings (sync / scalar) for input BW
            (nc.sync if ci % 2 == 0 else nc.scalar).dma_start(
                raw, src.rearrange("(q p) w -> p q w", p=128))
            state[ci] = {"raw": raw}

        def stage_vertical(ci):
            raw = state[ci]["raw"]
            hps = []
            for half in range(2):
                hp = hp_pool.tile(shape=[128, 512], dtype=F32, space="PSUM", name="hp")
                qs = mh_blocks[half]
                for qi, q in enumerate(qs):
                    nc.tensor.matmul(
                        hp, mh_sb[:, q, 128*half:128*(half+1)], raw[:, q, :],
                        start=(qi == 0), stop=(qi == len(qs) - 1),
                    )
                hps.append(hp)
            state[ci]["hps"] = hps

        def stage_prep(ci):
            hps = state[ci]["hps"]
            hs = wk_pool.tile(shape=[128, 2, 512], dtype=F32, name="hs")
            # PSUM -> SBUF (one per engine to balance ACT/DVE)
            ns.copy(hs[:, 0], hps[0])
            nv.tensor_copy(hs[:, 1], hps[1])

            # horizontal resize: v = 16*y in [0, 16]
            v = wk_pool.tile(shape=[128, 2, 256], dtype=F32, name="v")
            a2 = wk_pool.tile(shape=[128, 2, 256], dtype=F32, name="a2")
            b2 = wk_pool.tile(shape=[128, 2, 256], dtype=F32, name="b2")
            hsr = hs.rearrange("p h (i two) -> p h i two", two=2)
            ev, od = hsr[:, :, :, 0], hsr[:, :, :, 1]
            nv.tensor_tensor(a2, ev, od, ALU.add)                      # [128,2,256]
            nv.tensor_tensor(b2[:, :, 1:255], od[:, :, 0:254], ev[:, :, 2:256], ALU.add)
            nv.scalar_tensor_tensor(v[:, :, 1:255], a2[:, :, 1:255], 3.0,
                                    b2[:, :, 1:255], ALU.mult, ALU.add)
            for half in range(2):
                # v[0] = (8/7)*(3*a2[0] + hs[2]); v[255] = (8/7)*(3*a2[255] + hs[509])
                nv.scalar_tensor_tensor(v[:, half, 0:1], a2[:, half, 0:1], 3.0,
                                        hs[:, half, 2:3], ALU.mult, ALU.add)
                nv.tensor_scalar(v[:, half, 0:1], v[:, half, 0:1], 8.0/7.0, None, ALU.mult)
                nv.scalar_tensor_tensor(v[:, half, 255:256], a2[:, half, 255:256], 3.0,
                                        hs[:, half, 509:510], ALU.mult, ALU.add)
                nv.tensor_scalar(v[:, half, 255:256], v[:, half, 255:256], 8.0/7.0,
                                 None, ALU.mult)

            vf = v.rearrange("p h i -> p (h i)")                       # [128, 512]
            # h = round(v) in [0, 16] (DVE f32->i32 convert rounds to nearest)
            h32 = wk_pool.tile(shape=[128, 512], dtype=I32, name="h32")
            nv.tensor_copy(h32, vf)
            hbf = wk_pool.tile(shape=[128, 512], dtype=BF16, name="hbf")
            nv.tensor_copy(hbf, h32)
            # wv = v - h in [-0.5, 0.5] (mixed f32/bf16 operands; h exact in bf16)
            wv = wk_pool.tile(shape=[128, 512], dtype=F32, name="wv")
            nv.tensor_tensor(wv, vf, hbf, ALU.subtract)

            # basis columns Phi [128, NCOL, 512] bf16
            phi = phi_pool.tile(shape=[128, NCOL, 512], dtype=BF16, name="phi")
            for r in range(N_A):
                ns.activation(phi[:, r, :], wv, ACT.Tanh,
                              bias=bias_sb[:, r:r+1], scale=float(8.0*C))
            wn = wk_pool.tile(shape=[128, 512], dtype=BF16, name="wn")
            nv.tensor_scalar(wn, wv, 2.0, None, ALU.mult)
            nv.tensor_copy(phi[:, N_A, :], wn)
            for p in range(1, N_POW):
                nv.tensor_tensor(phi[:, N_A+p, :], phi[:, N_A+p-1, :], wn, ALU.mult)
            nv.memset(phi[:, NCOL-1, :], 1.0)

            # one-hot [128, 17, 512] bf16 (DVE only: GpSimd shares the SBUF
            # port with DVE under an exclusive lock — offloading there slows
            # both engines ~10x)
            oh = oh_pool.tile(shape=[128, N_M, 512], dtype=BF16, name="oh")
            for m in range(N_M):
                nv.tensor_scalar(oh[:, m, :], hbf, float(m), None, ALU.is_equal)
            state[ci]["phi"] = phi
            state[ci]["oh"] = oh

        def stage_scatter(ci):
            phi, oh = state[ci]["phi"], state[ci]["oh"]
            # scatter: GP=8 chunks per matmul.  Group g packs chunks
            # {g, g+64, .., g+448}: with columns ordered (c outer, j inner)
            # the address c*512 + 64*j is affine with stride 64, so both
            # operands collapse to a single legal free dim while phi/oh keep
            # their fast-write layouts.  Output cell (8c+j, 8m+j') accumulates
            # phi_c^T oh_m of chunk pair (j, j'); only the j==j' diagonal is
            # read on the host.
            aps = at_pool.tile(shape=[128, 512], dtype=F32, space="PSUM", name="aps")
            n_groups = 512 // GP
            for g in range(n_groups):
                st = phi[:, :, g::n_groups].rearrange("p c j -> p (c j)")
                mv = oh[:, :, g::n_groups].rearrange("p m j -> p (m j)")
                nc.tensor.matmul(aps[0:GP*NCOL, 0:GP*N_M], st, mv,
                                 start=(g == 0), stop=(g == n_groups - 1))
            stage = wk_pool.tile(shape=[GP*NCOL, GP*N_M], dtype=F32, name="stage")
            ns.copy(stage, aps[0:GP*NCOL, 0:GP*N_M])
            (nc.sync if ci % 2 == 0 else nc.scalar).dma_start(out[ci], stage)
            state.pop(ci)

        # software pipeline: vertical MMs for channel i+1 are emitted before
        # channel i's scatter so the PE never sits idle waiting on the DVE/ACT
        # prep chain (idle gaps re-throttle the PE clock gate to 1.2 GHz).
        for i in range(n_ch + 2):
            if i < n_ch:
                stage_dma(i)
                stage_vertical(i)
            if 0 <= i - 1 < n_ch:
                stage_prep(i - 1)
            if 0 <= i - 2 < n_ch:
                stage_scatter(i - 2)

        ch_ctx.close()
        ctx.close()

    nc.compile()
    return nc


_CACHE: dict = {}
LAST_RESULT = None


def _get_nc(n_pairs=6):
    if n_pairs not in _CACHE:
        _CACHE[n_pairs] = build(n_pairs)
    return _CACHE[n_pairs]


def kernel(pred: np.ndarray, target: np.ndarray) -> np.ndarray:
    global LAST_RESULT
    pred = np.ascontiguousarray(pred, dtype=np.float32)
    target = np.ascontiguousarray(target, dtype=np.float32)
    assert pred.shape == (16, 3, 512, 512) and target.shape == (16, 3, 512, 512)

    nc = _get_nc(6)
    mh_buf = make_mh2()
    in_maps = []
    for i in range(N_CORES):
        in_maps.append({
            "pred": pred[2*i:2*i+2],
            "target": target[2*i:2*i+2],
            "mh": mh_buf,
        })
    trace = os.environ.get("KERNEL_TRACE", "0") == "1"
    res = run_bass_kernel_spmd(nc, in_maps, core_ids=list(range(N_CORES)), trace=trace)
    LAST_RESULT = res

    # host-side fold: A[col, m] -> T(tp) -> CDF -> loss (float64)
    GP = 8
    L = host_fit()                                    # [NCOL, 513]
    tps = np.arange(257)
    losses = []
    for i in range(N_CORES):
        a = res.results[i]["out"].astype(np.float64)  # [12, GP*NCOL, GP*17]
        a4 = a.reshape(12, NCOL, GP, N_M, GP)
        A = np.diagonal(a4, axis1=2, axis2=4).sum(axis=-1)   # [12, NCOL, 17]
        cdfs = []
        for ch in range(12):
            T = np.zeros(257)
            for m in range(N_M):
                T += (A[ch, :, m][:, None] * L[:, tps - 16*m + 256]).sum(axis=0)
            Cn = T[0] - T[1:]
            cdfs.append(Cn / Cn[-1])
        for p in range(6):
            losses.append(np.mean(np.abs(cdfs[p] - cdfs[p+6])))
    return np.float32(np.mean(losses))


# revision 22
# speedup vs baseline: 2.6267x; 1.1153x over previous
"""Trainium2 Bass kernel for nn_CDFL1HistogramLoss (CDF-L1 histogram loss).

Math (see reference): per channel (16->256 resized), the CDF numerators
telescope to T(t) = sum_x sig(C*(u_x - t)), C = SIGMA/256, u = 256*x.
Decompose u = 16*m + w (m = round(u/16) in [0,16], w in [-8,8]).  Each
pixel's sigmoid profile sig(C*(w - (t - 16m))) is expanded in a small
per-pixel basis (tanh anchors from ScalarE + w-powers from VectorE); the
device scatters basis columns by coarse bucket m with a PE one-hot
matmul (stationary = basis columns Phi [128, NCOL], moving = one-hot
[128, 17], PSUM-accumulated over all 512 pixel chunks in 4 PE column
strips).  The host folds A[m, col] through a least-squares fit L of
sig(C*(w - trel)) in the (bf16-quantized) device basis to get T, the
CDFs, and the loss in float64.

Sharding: data-parallel over batch: core i handles batches [2i, 2i+1]
(12 channel-histograms = 6 pred + 6 target per core).
"""
import os
import numpy as np

import concourse.bass as bass
import concourse.bacc as bacc
import concourse.mybir as mybir
from concourse import tile
from concourse.bass_utils import run_bass_kernel_spmd

F32 = mybir.dt.float32
BF16 = mybir.dt.bfloat16
I32 = mybir.dt.int32
ALU = mybir.AluOpType
ACT = mybir.ActivationFunctionType

N_CORES = 8
SIGMA = 300.0
C = SIGMA / 256.0             # z-units per fine bin (1.171875)
N_M = 17                      # coarse buckets m = round(u/16) in [0, 16]
NPIX = 65536

# ---- basis definition (device + host must agree) ----
ANCHORS = (-10.5, -7.5, -4.5, -1.5, 1.5, 4.5, 7.5, 10.5)   # fine-bin offsets
N_POW = 3                     # wn, wn^2, wn^3 with wn = w/8 in [-1, 1]
N_A = len(ANCHORS)
NCOL = N_A + N_POW + 1        # + ones column (counts); must be <= 32


def bf16_rne(x):
    f = np.asarray(x, np.float32)
    u = f.view(np.uint32)
    r = ((u >> 16) & 1) + 0x7FFF
    return ((u + r) & 0xFFFF0000).view(np.float32)


def make_mh2() -> np.ndarray:
    """[512, 256] vertical resize matrix x2 (so v = 3a + b needs no extra scale)."""
    M = np.zeros((512, 256), dtype=np.float64)
    for i in range(256):
        if i == 0:
            M[0, 0], M[1, 0], M[2, 0] = 3/7, 3/7, 1/7
        elif i == 255:
            M[509, 255], M[510, 255], M[511, 255] = 1/7, 3/7, 3/7
        else:
            M[2*i-1, i] = 1/8; M[2*i, i] = 3/8; M[2*i+1, i] = 3/8; M[2*i+2, i] = 1/8
    return (2.0 * M).astype(np.float32)


def device_basis(wv):
    """Device-accurate basis columns for wv (f32 array in [-0.5, 0.5]).

    Column order matches the Phi SBUF tile rows:
      0..N_A-1 : bf16(tanh(8C*wv - (C/2)*t_r))
      N_A + p  : wn^(p+1) chain in bf16, wn = bf16(2*wv)
      NCOL-1   : ones
    """
    wv = np.asarray(wv, np.float32)
    cols = []
    for t in ANCHORS:
        cols.append(bf16_rne(np.tanh(np.float32(8.0*C)*wv + np.float32(-(C/2.0)*t),
                                     dtype=np.float64).astype(np.float32)))
    wn = bf16_rne(2.0*wv)
    p = wn
    cols.append(p)
    for _ in range(N_POW - 1):
        p = bf16_rne(p * wn)
        cols.append(p)
    cols.append(np.ones_like(wv))
    return np.stack(cols, axis=-1)   # [..., NCOL]


def host_fit():
    """L[col, trel+256] fitting sig(C*(16*wv - trel)) in the device basis."""
    wv = np.linspace(-0.5, 0.5, 8193).astype(np.float32)
    B = device_basis(wv).astype(np.float64)
    trels = np.arange(-256, 257)
    G = 1.0/(1.0 + np.exp(-C*(16.0*wv.astype(np.float64)[:, None] - trels[None, :])))
    L, *_ = np.linalg.lstsq(B, G, rcond=None)
    return L                      # [NCOL, 513]


def build(n_pairs: int = 6):
    """Per-core Bass program: n_pairs pred + n_pairs target channels."""
    MH2 = make_mh2()
    # nonzero 128x128 blocks of MH2 per output half
    mh_blocks = {}
    for half in range(2):
        qs = []
        for q in range(4):
            if np.any(MH2[128*q:128*(q+1), 128*half:128*(half+1)] != 0):
                qs.append(q)
        mh_blocks[half] = qs
    n_ch = 2 * n_pairs

    GP = 8                      # chunks packed per scatter matmul (diagonal blocks)
    nc = bacc.Bacc("TRN2", target_bir_lowering=False, debug=False, num_devices=N_CORES)
    pred = nc.dram_tensor("pred", [2, 3, 512, 512], F32, kind="ExternalInput").ap()
    target = nc.dram_tensor("target", [2, 3, 512, 512], F32, kind="ExternalInput").ap()
    mh = nc.dram_tensor("mh", [512, 256], BF16, kind="ExternalInput").ap()
    out = nc.dram_tensor("out", [n_ch, GP * NCOL, GP * N_M], F32,
                         kind="ExternalOutput").ap()

    with tile.TileContext(nc) as tc:
        from contextlib import ExitStack
        nv = nc.vector
        ns = nc.scalar
        ctx = ExitStack()
        cpool = ctx.enter_context(tc.tile_pool(name="consts", bufs=1))

        mh_sb = cpool.tile(shape=[128, 4, 256], dtype=BF16, name="mh_sb")
        nc.sync.dma_start(mh_sb, mh.rearrange("(q p) w -> p q w", p=128))
        # bias_sb[:, r] = -(C/2)*ANCHORS[r]  (anchors uniform: -10.5 + 3r)
        iota_i = cpool.tile(shape=[128, 32], dtype=I32, name="iota_i")
        nc.gpsimd.iota(iota_i, pattern=[[1, 32]], base=0, channel_multiplier=0)
        bias_sb = cpool.tile(shape=[128, N_A], dtype=F32, name="bias_sb")
        nv.tensor_scalar(bias_sb, iota_i[:, 0:N_A], float(-3.0*(C/2.0)),
                         float(10.5*(C/2.0)), ALU.mult, ALU.add)

        ch_ctx = ExitStack()
        io_pool = ch_ctx.enter_context(tc.tile_pool(name="io", bufs=3))
        wk_pool = ch_ctx.enter_context(tc.tile_pool(name="wk", bufs=2))
        phi_pool = ch_ctx.enter_context(tc.tile_pool(name="phi", bufs=2))
        oh_pool = ch_ctx.enter_context(tc.tile_pool(name="oh", bufs=2))
        hp_pool = ch_ctx.enter_context(tc.tile_pool(name="hp", bufs=4, space="PSUM"))
        at_pool = ch_ctx.enter_context(tc.tile_pool(name="at", bufs=2, space="PSUM"))

        chans = [("p", i) for i in range(n_pairs)] + [("t", i) for i in range(n_pairs)]
        state: dict = {}

        def stage_dma(ci):
            grp, pi = chans[ci]
            b, cch = divmod(pi, 3)
            src = (pred if grp == "p" else target)[b, cch]      # [512, 512] dram
            raw = io_pool.tile(shape=[128, 4, 512], dtype=F32, name="raw")
            # alternate the two HWDGE rings (sync / scalar) for input BW
            (nc.sync if ci % 2 == 0 else nc.scalar).dma_start(
                raw, src.rearrange("(q p) w -> p q w", p=128))
            state[ci] = {"raw": raw}

        def stage_vertical(ci):
            # cast raw to bf16: fp32 PE matmuls run as two hidden passes (HI/LO)
            # and block FWL; bf16 input noise shifts the loss by only ~7e-4 rel.
            raw = state[ci]["raw"]
            raw_bf = io_pool.tile(shape=[128, 4, 512], dtype=BF16, name="raw_bf")
            nv.tensor_copy(raw_bf, raw)
            hps = []
            for half in range(2):
                hp = hp_pool.tile(shape=[128, 512], dtype=F32, space="PSUM", name="hp")
                qs = mh_blocks[half]
                for qi, q in enumerate(qs):
                    nc.tensor.matmul(
                        hp, mh_sb[:, q, 128*half:128*(half+1)], raw_bf[:, q, :],
                        start=(qi == 0), stop=(qi == len(qs) - 1),
                    )
                hps.append(hp)
            state[ci]["hps"] = hps

        def stage_prep(ci):
            hps = state[ci]["hps"]
            hs = wk_pool.tile(shape=[128, 2, 512], dtype=F32, name="hs")
            # PSUM -> SBUF (one per engine to balance ACT/DVE)
            ns.copy(hs[:, 0], hps[0])
            nv.tensor_copy(hs[:, 1], hps[1])

            # horizontal resize: v = 16*y in [0, 16]
            v = wk_pool.tile(shape=[128, 2, 256], dtype=F32, name="v")
            a2 = wk_pool.tile(shape=[128, 2, 256], dtype=F32, name="a2")
            b2 = wk_pool.tile(shape=[128, 2, 256], dtype=F32, name="b2")
            hsr = hs.rearrange("p h (i two) -> p h i two", two=2)
            ev, od = hsr[:, :, :, 0], hsr[:, :, :, 1]
            nv.tensor_tensor(a2, ev, od, ALU.add)                      # [128,2,256]
            nv.tensor_tensor(b2[:, :, 1:255], od[:, :, 0:254], ev[:, :, 2:256], ALU.add)
            nv.scalar_tensor_tensor(v[:, :, 1:255], a2[:, :, 1:255], 3.0,
                                    b2[:, :, 1:255], ALU.mult, ALU.add)
            for half in range(2):
                # v[0] = (8/7)*(3*a2[0] + hs[2]); v[255] = (8/7)*(3*a2[255] + hs[509])
                nv.scalar_tensor_tensor(v[:, half, 0:1], a2[:, half, 0:1], 3.0,
                                        hs[:, half, 2:3], ALU.mult, ALU.add)
                nv.tensor_scalar(v[:, half, 0:1], v[:, half, 0:1], 8.0/7.0, None, ALU.mult)
                nv.scalar_tensor_tensor(v[:, half, 255:256], a2[:, half, 255:256], 3.0,
                                        hs[:, half, 509:510], ALU.mult, ALU.add)
                nv.tensor_scalar(v[:, half, 255:256], v[:, half, 255:256], 8.0/7.0,
                                 None, ALU.mult)

            vf = v.rearrange("p h i -> p (h i)")                       # [128, 512]
            # h = round(v) in [0, 16] (DVE f32->i32 convert rounds to nearest)
            h32 = wk_pool.tile(shape=[128, 512], dtype=I32, name="h32")
            nv.tensor_copy(h32, vf)
            hbf = wk_pool.tile(shape=[128, 512], dtype=BF16, name="hbf")
            nv.tensor_copy(hbf, h32)
            # wv = v - h in [-0.5, 0.5] (mixed f32/bf16 operands; h exact in bf16)
            wv = wk_pool.tile(shape=[128, 512], dtype=F32, name="wv")
            nv.tensor_tensor(wv, vf, hbf, ALU.subtract)

            # basis columns Phi [128, NCOL, 512] bf16
            phi = phi_pool.tile(shape=[128, NCOL, 512], dtype=BF16, name="phi")
            for r in range(N_A):
                ns.activation(phi[:, r, :], wv, ACT.Tanh,
                              bias=bias_sb[:, r:r+1], scale=float(8.0*C))
            wn = wk_pool.tile(shape=[128, 512], dtype=BF16, name="wn")
            nv.tensor_scalar(wn, wv, 2.0, None, ALU.mult)
            nv.tensor_copy(phi[:, N_A, :], wn)
            for p in range(1, N_POW):
                nv.tensor_tensor(phi[:, N_A+p, :], phi[:, N_A+p-1, :], wn, ALU.mult)
            nv.memset(phi[:, NCOL-1, :], 1.0)

            # one-hot [128, 17, 512] bf16 (DVE only: GpSimd shares the SBUF
            # port with DVE under an exclusive lock — offloading there slows
            # both engines ~10x)
            oh = oh_pool.tile(shape=[128, N_M, 512], dtype=BF16, name="oh")
            for m in range(N_M):
                nv.tensor_scalar(oh[:, m, :], hbf, float(m), None, ALU.is_equal)
            state[ci]["phi"] = phi
            state[ci]["oh"] = oh

        def stage_scatter(ci):
            phi, oh = state[ci]["phi"], state[ci]["oh"]
            # scatter: GP=8 chunks per matmul.  Group g packs chunks
            # {g, g+64, .., g+448}: with columns ordered (c outer, j inner)
            # the address c*512 + 64*j is affine with stride 64, so both
            # operands collapse to a single legal free dim while phi/oh keep
            # their fast-write layouts.  Output cell (8c+j, 8m+j') accumulates
            # phi_c^T oh_m of chunk pair (j, j'); only the j==j' diagonal is
            # read on the host.
            aps = at_pool.tile(shape=[128, 512], dtype=F32, space="PSUM", name="aps")
            n_groups = 512 // GP
            for g in range(n_groups):
                st = phi[:, :, g::n_groups].rearrange("p c j -> p (c j)")
                mv = oh[:, :, g::n_groups].rearrange("p m j -> p (m j)")
                nc.tensor.matmul(aps[0:GP*NCOL, 0:GP*N_M], st, mv,
                                 start=(g == 0), stop=(g == n_groups - 1))
            stage = wk_pool.tile(shape=[GP*NCOL, GP*N_M], dtype=F32, name="stage")
            ns.copy(stage, aps[0:GP*NCOL, 0:GP*N_M])
            (nc.sync if ci % 2 == 0 else nc.scalar).dma_start(out[ci], stage)
            state.pop(ci)

        # software pipeline: DMA leads by 2 iterations so the bf16 cast and
        # vertical MMs never stall on an in-flight transfer; vertical+prep for
        # channel k+1 are emitted before channel k's scatter so the PE never
        # sits idle waiting on the DVE/ACT prep chain (idle gaps re-throttle
        # the PE clock gate to 1.2 GHz).
        for i in range(n_ch + 3):
            if i < n_ch:
                stage_dma(i)
            if 0 <= i - 2 < n_ch:
                stage_vertical(i - 2)
                stage_prep(i - 2)
            if 0 <= i - 3 < n_ch:
                stage_scatter(i - 3)

        ch_ctx.close()
        ctx.close()

    nc.compile()
    return nc


_CACHE: dict = {}
LAST_RESULT = None


def _get_nc(n_pairs=6):
    if n_pairs not in _CACHE:
        _CACHE[n_pairs] = build(n_pairs)
    return _CACHE[n_pairs]


def kernel(pred: np.ndarray, target: np.ndarray) -> np.ndarray:
    global LAST_RESULT
    pred = np.ascontiguousarray(pred, dtype=np.float32)
    target = np.ascontiguousarray(target, dtype=np.float32)
    assert pred.shape == (16, 3, 512, 512) and target.shape == (16, 3, 512, 512)

    import ml_dtypes
    nc = _get_nc(6)
    mh_buf = make_mh2().astype(ml_dtypes.bfloat16)
    in_maps = []
    for i in range(N_CORES):
        in_maps.append({
            "pred": pred[2*i:2*i+2],
            "target": target[2*i:2*i+2],
            "mh": mh_buf,
        })
    trace = os.environ.get("KERNEL_TRACE", "0") == "1"
    res = run_bass_kernel_spmd(nc, in_maps, core_ids=list(range(N_CORES)), trace=trace)
    LAST_RESULT = res

    # host-side fold: A[col, m] -> T(tp) -> CDF -> loss (float64)
    GP = 8
    L = host_fit()                                    # [NCOL, 513]
    tps = np.arange(257)
    losses = []
    for i in range(N_CORES):
        a = res.results[i]["out"].astype(np.float64)  # [12, GP*NCOL, GP*17]
        a4 = a.reshape(12, NCOL, GP, N_M, GP)
        A = np.diagonal(a4, axis1=2, axis2=4).sum(axis=-1)   # [12, NCOL, 17]
        cdfs = []
        for ch in range(12):
            T = np.zeros(257)
            for m in range(N_M):
                T += (A[ch, :, m][:, None] * L[:, tps - 16*m + 256]).sum(axis=0)
            Cn = T[0] - T[1:]
            cdfs.append(Cn / Cn[-1])
        for p in range(6):
            losses.append(np.mean(np.abs(cdfs[p] - cdfs[p+6])))
    return np.float32(np.mean(losses))
